# revision 8
# baseline (speedup 1.0000x reference)
"""BoundaryMaxPooling Trainium2 kernel (v2: DVE sparse table + gpsimd gather).

Reference computation (B=16, C2=512, T=Tf=126):
  - segment windows [s0,s1) / [e0,e1) derived from segments[0] only
  - out[b, c, t]      = max_{j in [s0(t), s1(t))} feature[b, c, j]       (c < 256)
  - out[b, 256+c, t]  = max_{j in [e0(t), e1(t))} feature[b, 256+c, t]

Device algorithm (per core, 2 batches, data-parallel over batch):
  Layout: channels on SBUF partitions, time j on the free axis, bf16.
  Per half h (start/end windows) a sparse table L_k[j] (range max of
  length 2^k starting at j) is built with 6 DVE tensor_max ops over
  multi-dim affine APs (both halves in one op per level, 2x bf16 mode).
  The query out[t] = max(L_k[lo], L_k[hi - 2^k]) (k = floor(log2 len))
  is an irregular free-axis gather: done by one gpsimd ap_gather per
  half using host-computed int16 column indices (runtime data, so the
  module compiles once).  A final DVE tensor_max combines the two
  gathered operands.  No PE, no ACT engine; total DMA ~0.5 MB/core.
  Host pre/post-processing is layout permutation + bf16 cast only;
  empty end-windows are data-independent and filled with f32 min on
  the host, matching the reference.
"""

import os
import sys

import numpy as np
import ml_dtypes

if os.path.isdir("/opt/trn_rl_repo") and "/opt/trn_rl_repo" not in sys.path:
    sys.path.insert(0, "/opt/trn_rl_repo")

import concourse.bass as bass  # noqa: E402
from concourse import bacc, mybir, tile  # noqa: E402
from concourse.bass_utils import run_bass_kernel_spmd  # noqa: E402

B, C2, T = 16, 512, 126
C = C2 // 2  # 256
NCORES = 8
BPC = B // NCORES  # batches per core = 2
NLEV = 7
KS = [127 - (1 << k) for k in range(NLEV)]  # level lengths: 126,125,...,63
OFFS = [0]
for _k in range(NLEV - 1):
    OFFS.append(OFFS[-1] + KS[_k])
TOTC = OFFS[-1] + KS[-1]  # 762 columns per half
G = 4  # channel groups per half (4 * 128 partitions = 512 ch)
HSTRIDE = TOTC * G  # 3048 elems per half
NIDX = 256  # idxs per gather call (126 A + 126 B + 4 pad)

BF16 = mybir.dt.bfloat16
I16 = mybir.dt.int16

_CACHE = {}

TRACE = False
LAST_RESULTS = None


def _build_module():
    nc = bacc.Bacc(None, target_bir_lowering=False, debug=False)

    ft = nc.dram_tensor("ft", [128, 2 * 126 * G], BF16, kind="ExternalInput")
    ix = nc.dram_tensor("ix", [128, 36], I16, kind="ExternalInput")
    out = nc.dram_tensor("out", [128, T * 2 * G], BF16, kind="ExternalOutput")

    with tile.TileContext(nc) as tc:
        with tc.tile_pool(name="mn", bufs=1) as pool:
            big = pool.tile([128, 2 * HSTRIDE], BF16, name="big")
            gt = pool.tile([128, 2 * NIDX * G], BF16, name="gt")
            ot = pool.tile([128, T * 2 * G], BF16, name="ot")
            ixt = pool.tile([128, 36], I16, name="ixt")
            scr = pool.tile([128, 64], I16, name="scr")
            dscr = pool.tile([128, 20], I16, name="dscr")

            bigR = big[:, :].rearrange("p (h c g) -> p h c g", h=2, c=TOTC, g=G)
            gtR = gt[:, :].rearrange("p (h n g) -> p h n g", h=2, n=NIDX, g=G)
            otR = ot[:, :].rearrange("p (t h g) -> p t h g", t=T, h=2, g=G)

            # input DMAs: feature into the L0 region of each half, idxs
            nc.gpsimd.dma_start(out=ixt[:, :], in_=ix[:, :])
            nc.sync.dma_start(out=bigR[:, :, 0:126, :], in_=ft[:, :])

            # dummy ap_gather so the gpsimd library load + first-extended-
            # instruction overhead happen while the feature DMA is still in
            # flight; its zero indices come from a DVE memset, so it has no
            # DMA dependency at all
            nc.vector.memset(dscr[:, :], 0)
            nc.gpsimd.ap_gather(
                scr[:, 0:64].rearrange("p (n d) -> p n d", n=16, d=4),
                dscr[:, 4:20].rearrange("p (n d) -> p n d", n=4, d=4),
                dscr[:, 0:1],
                channels=128,
                num_elems=4,
                d=4,
                num_idxs=16,
            )

            # sparse-table levels: L_{k+1}[j] = max(L_k[j], L_k[j + 2^k])
            for k in range(NLEV - 1):
                n = KS[k + 1]
                o0, o1 = OFFS[k], OFFS[k + 1]
                nc.vector.tensor_max(
                    bigR[:, :, o1 : o1 + n, :],
                    bigR[:, :, o0 : o0 + n, :],
                    bigR[:, :, o0 + (1 << k) : o0 + (1 << k) + n, :],
                )

            # query gather: ONE ap_gather over the combined half tables
            # (half-1 columns live at +TOTC in the concatenated space)
            nc.gpsimd.ap_gather(
                gt[:, :].rearrange("p (n g) -> p n g", n=2 * NIDX, g=G),
                big[:, :].rearrange("p (c g) -> p c g", c=2 * TOTC, g=G),
                ixt[:, 0:32],
                channels=128,
                num_elems=2 * TOTC,
                d=G,
                num_idxs=2 * NIDX,
            )

            # final: out[t] = max(gathered_A[t], gathered_B[t])
            for h in range(2):
                nc.vector.tensor_max(
                    otR[:, :, h, :],
                    gtR[:, h, 0:T, :],
                    gtR[:, h, T : 2 * T, :],
                )

            nc.gpsimd.dma_start(out=out[:, :], in_=ot[:, :])

    nc.compile()
    return nc


def _host_windows(segments):
    """Replicates the reference's index math on segments[0]. Returns per half
    (lo, hi) clamped windows plus the empty mask."""
    seg = np.clip(segments.astype(np.float32), 0.0, 125.0)
    row = seg[0]  # [T, 4]
    s0 = np.floor(row[:, 0]).astype(np.int32)
    s1 = np.ceil(row[:, 1]).astype(np.int32)
    s1 = np.where(s0 == s1, s1 + 1, s1)
    e0 = np.floor(row[:, 2]).astype(np.int32)
    e1 = np.ceil(row[:, 3]).astype(np.int32)
    e0 = np.where(e0 == e1, e0 - 1, e0)

    halves = []
    for lo, hi in ((s0, s1), (e0, e1)):
        lo_c = np.maximum(lo, 0)
        hi_c = np.minimum(hi, T)
        empty = lo_c >= hi_c
        halves.append((lo_c, hi_c, empty))
    return halves


def _host_idxs(halves):
    """int16 gather-index tensor [128, 36]: half h occupies cols
    [16h, 16h+16) (4-byte aligned); col 32 is zeros for the dummy gather."""
    ix = np.zeros((128, 36), np.int16)
    for h, (lo, hi, empty) in enumerate(halves):
        ln = hi - lo
        ln = np.maximum(ln, 1)  # empty windows -> dummy idx 0
        k = np.frexp(ln.astype(np.float64))[1] - 1  # floor(log2(len))
        offk = np.array(OFFS, np.int32)[k]
        col_a = offk + lo
        col_b = offk + hi - (1 << k)
        col_a = np.where(empty, 0, col_a) + h * TOTC
        col_b = np.where(empty, 0, col_b) + h * TOTC
        idx_list = np.zeros(NIDX, np.int16)
        idx_list[0:T] = col_a
        idx_list[T : 2 * T] = col_b
        wrapped = idx_list.reshape(16, 16).T  # [p_, s]: m = s*16 + p_
        ix[:, h * 16 : h * 16 + 16] = np.tile(wrapped, (8, 1))
    return ix


def _host_feature(feature, core):
    """[128, 1008] bf16 for one core: layout [p][h, j, g], g = b_local*2+ch2,
    channel = h*256 + ch2*128 + p."""
    fc = feature[BPC * core : BPC * (core + 1)]  # [2, 512, 126] f32
    arr = fc.reshape(BPC, 2, 2, 128, T)  # [b, h, ch2, p, j]
    arr = arr.transpose(3, 1, 4, 0, 2)  # [p, h, j, b, ch2]
    return np.ascontiguousarray(arr.reshape(128, 2 * T * G)).astype(
        ml_dtypes.bfloat16
    )


def _unshard(results, halves):
    out = np.empty((B, C2, T), np.float32)
    for i in range(NCORES):
        r = np.asarray(results[i]["out"])  # [128, 1008] bf16
        arr = r.reshape(128, T, 2, BPC, 2).astype(np.float32)  # [p,t,h,b,ch2]
        arr = arr.transpose(3, 2, 4, 0, 1)  # [b, h, ch2, p, t]
        out[BPC * i : BPC * (i + 1)] = arr.reshape(BPC, C2, T)
    neg = np.finfo(np.float32).min
    for h, (_, _, empty) in enumerate(halves):
        if empty.any():
            out[:, h * C : (h + 1) * C, empty] = neg
    return out


def kernel(feature, segments):
    global LAST_RESULTS
    feature = np.ascontiguousarray(feature, dtype=np.float32)
    segments = np.ascontiguousarray(segments, dtype=np.float32)

    if "nc" not in _CACHE:
        _CACHE["nc"] = _build_module()
    nc = _CACHE["nc"]

    halves = _host_windows(segments)
    ix = _host_idxs(halves)
    in_maps = [
        {"ft": _host_feature(feature, i), "ix": ix} for i in range(NCORES)
    ]

    res = run_bass_kernel_spmd(nc, in_maps, list(range(NCORES)), trace=TRACE)
    LAST_RESULTS = res
    return _unshard(res.results, halves)


# revision 11
# speedup vs baseline: 1.1958x; 1.1958x over previous
"""BoundaryMaxPooling Trainium2 kernel (v3: bf16 PE sparse-table).

Reference computation (B=16, C2=512, T=Tf=126):
  - segment windows [s0,s1) / [e0,e1) derived from segments[0] only
  - out[b, c, t]      = max_{j in [s0(t), s1(t))} feature[b, c, j]       (c < 256)
  - out[b, 256+c, t]  = max_{j in [e0(t), e1(t))} feature[b, 256+c, j]

Device algorithm (per core, 2 batches, data-parallel over batch), all in
bf16 (rel tol 2e-2 >> bf16's 4e-3):
  Sparse-table (log-level) range max with time j on SBUF partitions:
    L_0[j, c'] = feature^T   (c' = half*512 + b*256 + c, 1024 columns)
    L_{k+1}[j] = max(L_k[j], L_k[j + 2^k])
  Partition shifts and the two per-window lookups (columns lo and
  hi - 2^k of level floor(log2 len)) are exact one-hot bf16 matmuls on
  the PE (1 cycle/row), accumulated over levels in f32 PSUM; host
  precomputes the one-hot matrices from segments[0] (replicated across
  cores).  Weights stream over four DMA queues ordered by consumption;
  a short run of zero matmuls keeps the PE busy from kernel start so
  HAM has it at full clock when the real data lands.  The final
  max(acc_a, acc_b) is one DVE op per half, PSUM -> SBUF bf16; host
  casts to f32, reassembles, and fills empty end-windows with f32 min
  (data-independent), matching the reference.
"""

import os
import sys

import numpy as np
import ml_dtypes

if os.path.isdir("/opt/trn_rl_repo") and "/opt/trn_rl_repo" not in sys.path:
    sys.path.insert(0, "/opt/trn_rl_repo")

import concourse.bass as bass  # noqa: E402
from concourse import bacc, mybir, tile  # noqa: E402
from concourse.bass_utils import run_bass_kernel_spmd  # noqa: E402

B, C2, T = 16, 512, 126
C = C2 // 2  # 256
NCORES = 8
BPC = B // NCORES  # batches per core = 2
CPRIME = BPC * C2  # 1024 columns per core
NLEV = 7
KS = [127 - (1 << k) for k in range(NLEV)]  # valid rows of level k

BF16 = mybir.dt.bfloat16
F32 = mybir.dt.float32

N_WARMUP = 5  # zero matmuls bridging kernel start -> first weights

_CACHE = {}

TRACE = False
LAST_RESULTS = None

# chunk c holds, per level k in CHUNK_LEVELS[c]: [sh_k | g(0,0,k) g(1,0,k)
# g(0,1,k) g(1,1,k)]  (sh only for k < 6)
CHUNK_LEVELS = [[0], [1, 2], [3, 4, 5, 6]]


def _wts_layout():
    """ft chunk + three weight chunks (one DMA each, ordered by level)."""
    offs = {}
    totals = []
    for levels in CHUNK_LEVELS:
        off = 0
        ci = len(totals) + 1
        for k in levels:
            if k < NLEV - 1:
                offs[("sh", k)] = (ci, off, KS[k + 1])
                off += KS[k + 1]
            for gi in range(2):
                for h in range(2):
                    offs[("g", gi, h, k)] = (ci, off, T)
                    off += T
        totals.append(off)
    return offs, totals


def _build_module():
    nc = bacc.Bacc(None, target_bir_lowering=False, debug=False)

    offs, totals = _wts_layout()
    ft = nc.dram_tensor("ft", [T, CPRIME], BF16, kind="ExternalInput")
    wts = [
        nc.dram_tensor(f"w{c}", [T, totals[c]], BF16, kind="ExternalInput")
        for c in range(3)
    ]
    out = nc.dram_tensor("out", [T, CPRIME], BF16, kind="ExternalOutput")

    with tile.TileContext(nc) as tc:
        with (
            tc.tile_pool(name="lv", bufs=1) as lvp,
            tc.tile_pool(name="gw", bufs=1) as gwp,
            tc.tile_pool(name="acc", bufs=1, space=bass.MemorySpace.PSUM) as accp,
            tc.tile_pool(name="shp", bufs=2, space=bass.MemorySpace.PSUM) as shpp,
        ):
            ftt = gwp.tile([T, CPRIME], BF16, name="ftt")
            wt = [
                gwp.tile([T, totals[c]], BF16, name=f"wt{c}") for c in range(3)
            ]
            # four queues: ft on SP, weight chunks on Pool/ACT/DVE queues,
            # ordered so level-0 weights land first.
            nc.sync.dma_start(out=ftt[:, :], in_=ft[:, :])
            nc.gpsimd.dma_start(out=wt[0][:, :], in_=wts[0][:, :])
            nc.scalar.dma_start(out=wt[1][:, :], in_=wts[1][:, :])
            nc.scalar.dma_start(out=wt[2][:, :], in_=wts[2][:, :])

            L = [ftt[:, :]] + [
                lvp.tile([KS[k], CPRIME], BF16, name=f"L{k}")[:, :]
                for k in range(1, NLEV)
            ]

            def sh_ap(k):
                ci, o, n = offs[("sh", k)]
                return wt[ci - 1][0 : KS[k], o : o + n]

            def g_ap(gi, h, k):
                ci, o, n = offs[("g", gi, h, k)]
                return wt[ci - 1][0 : KS[k], o : o + n]

            p_acc = [
                accp.tile([T, CPRIME], F32, name=f"pacc{gi}") for gi in range(2)
            ]

            # PE warmup: HAM runs the PE at reduced clock until ~3us of
            # continuous busy; zero bf16 matmuls bridge the gap between
            # kernel start and the first weight DMA landing so the real
            # matmuls run at full clock with no idle gap.
            wzero = gwp.tile([128, 512], BF16, name="wzero")
            nc.vector.memset(wzero[:, :], 0.0)
            for _ in range(N_WARMUP):
                nc.tensor.matmul(
                    p_acc[0][:, 0:512],
                    wzero[0:126, 0:126],
                    wzero[0:126, 0:512],
                    start=True,
                    stop=True,
                )

            # per level: shift both halves first (PE), gathers accumulate,
            # DVE folds the shifted rows into the next level meanwhile.
            for k in range(NLEV):
                shp = None
                if k < NLEV - 1:
                    shp = shpp.tile(
                        [KS[k + 1], CPRIME], F32, name=f"shp{k}", tag="shp"
                    )
                for h in range(2):
                    sl = slice(h * 512, (h + 1) * 512)
                    if k < NLEV - 1:
                        nc.tensor.matmul(
                            shp[:, sl],
                            sh_ap(k),
                            L[k][:, sl],
                            start=True,
                            stop=True,
                        )
                    for gi in range(2):
                        nc.tensor.matmul(
                            p_acc[gi][:, sl],
                            g_ap(gi, h, k),
                            L[k][:, sl],
                            start=(k == 0),
                            stop=(k == NLEV - 1),
                        )
                    if k < NLEV - 1:
                        nc.vector.tensor_max(
                            L[k + 1][:, sl],
                            L[k][0 : KS[k + 1], sl],
                            shp[:, sl],
                        )

            # final per half (DVE can read only one PSUM operand per op):
            # stage acc0 to SBUF bf16 via tensor_scalar_max, then max with
            # acc1; one output DMA per half.
            s1t = gwp.tile([T, CPRIME], BF16, name="s1t")
            ot = gwp.tile([T, CPRIME], BF16, name="ot")
            for half in range(2):
                sl = slice(half * 512, (half + 1) * 512)
                nc.vector.tensor_scalar_max(s1t[:, sl], p_acc[0][:, sl], -3.0e38)
                nc.vector.tensor_max(ot[:, sl], s1t[:, sl], p_acc[1][:, sl])
                eng = nc.sync if half == 0 else nc.gpsimd
                eng.dma_start(out=out[:, sl], in_=ot[:, sl])

    nc.compile()
    return nc


def _host_windows(segments):
    """Replicates the reference's index math on segments[0]. Returns per half
    (lo, hi) clamped windows plus the empty mask."""
    seg = np.clip(segments.astype(np.float32), 0.0, 125.0)
    row = seg[0]  # [T, 4]
    s0 = np.floor(row[:, 0]).astype(np.int32)
    s1 = np.ceil(row[:, 1]).astype(np.int32)
    s1 = np.where(s0 == s1, s1 + 1, s1)
    e0 = np.floor(row[:, 2]).astype(np.int32)
    e1 = np.ceil(row[:, 3]).astype(np.int32)
    e0 = np.where(e0 == e1, e0 - 1, e0)

    halves = []
    for lo, hi in ((s0, s1), (e0, e1)):
        lo_c = np.maximum(lo, 0)
        hi_c = np.minimum(hi, T)
        empty = lo_c >= hi_c
        halves.append((lo_c, hi_c, empty))
    return halves


def _host_matrices(segments):
    halves = _host_windows(segments)
    g = {
        (gi, h, k): np.zeros((KS[k], T), np.float32)
        for gi in range(2)
        for h in range(2)
        for k in range(NLEV)
    }
    for h, (lo, hi, empty) in enumerate(halves):
        for t in range(T):
            if empty[t]:
                continue
            ln = int(hi[t] - lo[t])
            k = ln.bit_length() - 1
            a = int(lo[t])
            b = int(hi[t]) - (1 << k)
            g[(0, h, k)][a, t] = 1.0
            g[(1, h, k)][b, t] = 1.0
    sh = {}
    for k in range(NLEV - 1):
        m = np.zeros((KS[k], KS[k + 1]), np.float32)
        s = 1 << k
        for j in range(KS[k + 1]):
            m[j + s, j] = 1.0
        sh[k] = m
    return g, sh, halves


def _shard_feature(feature):
    """Core i gets batches [2i, 2i+2) as [T, CPRIME] bf16 with
    c' = half*512 + local_batch*256 + channel_within_half."""
    fts = []
    for i in range(NCORES):
        pair = feature[BPC * i : BPC * (i + 1)]
        arr = pair.reshape(BPC, 2, C, T)  # [b, h, c, j]
        arr = np.ascontiguousarray(arr.transpose(3, 1, 0, 2).reshape(T, CPRIME))
        fts.append(arr.astype(ml_dtypes.bfloat16))
    return fts


def _unshard(results, halves):
    out = np.empty((B, C2, T), np.float32)
    for i in range(NCORES):
        r = np.asarray(results[i]["out"]).astype(np.float32)  # [T, CPRIME]
        arr = r.reshape(T, 2, BPC, C).transpose(2, 1, 3, 0)  # [b, h, c, t]
        out[BPC * i : BPC * (i + 1)] = arr.reshape(BPC, C2, T)
    neg = np.finfo(np.float32).min
    for h, (_, _, empty) in enumerate(halves):
        if empty.any():
            out[:, h * C : (h + 1) * C, empty] = neg
    return out


def kernel(feature, segments):
    global LAST_RESULTS
    feature = np.ascontiguousarray(feature, dtype=np.float32)
    segments = np.ascontiguousarray(segments, dtype=np.float32)

    if "nc" not in _CACHE:
        _CACHE["nc"] = _build_module()
    nc = _CACHE["nc"]

    g, sh, halves = _host_matrices(segments)
    fts = _shard_feature(feature)

    offs, totals = _wts_layout()
    chunks = [np.zeros((T, totals[c]), np.float32) for c in range(3)]
    for k in range(NLEV):
        if k < NLEV - 1:
            ci, o, n = offs[("sh", k)]
            chunks[ci - 1][: KS[k], o : o + n] = sh[k]
        for gi in range(2):
            for h in range(2):
                ci, o, n = offs[("g", gi, h, k)]
                chunks[ci - 1][: KS[k], o : o + n] = g[(gi, h, k)]
    chunks = [c.astype(ml_dtypes.bfloat16) for c in chunks]
    in_maps = []
    for i in range(NCORES):
        m = {"ft": fts[i]}
        for c in range(3):
            m[f"w{c}"] = chunks[c]
        in_maps.append(m)

    res = run_bass_kernel_spmd(nc, in_maps, list(range(NCORES)), trace=TRACE)
    LAST_RESULTS = res
    return _unshard(res.results, halves)


# revision 13
# speedup vs baseline: 1.2023x; 1.0055x over previous
"""BoundaryMaxPooling Trainium2 kernel (v3: bf16 PE sparse-table).

Reference computation (B=16, C2=512, T=Tf=126):
  - segment windows [s0,s1) / [e0,e1) derived from segments[0] only
  - out[b, c, t]      = max_{j in [s0(t), s1(t))} feature[b, c, j]       (c < 256)
  - out[b, 256+c, t]  = max_{j in [e0(t), e1(t))} feature[b, 256+c, j]

Device algorithm (per core, 2 batches, data-parallel over batch), all in
bf16 (rel tol 2e-2 >> bf16's 4e-3):
  Sparse-table (log-level) range max with time j on SBUF partitions:
    L_0[j, c'] = feature^T   (c' = half*512 + b*256 + c, 1024 columns)
    L_{k+1}[j] = max(L_k[j], L_k[j + 2^k])
  Partition shifts and the two per-window lookups (columns lo and
  hi - 2^k of level floor(log2 len)) are exact one-hot bf16 matmuls on
  the PE (1 cycle/row), accumulated over levels in f32 PSUM; host
  precomputes the one-hot matrices from segments[0] (replicated across
  cores).  Weights stream over four DMA queues ordered by consumption;
  a short run of zero matmuls keeps the PE busy from kernel start so
  HAM has it at full clock when the real data lands.  The final
  max(acc_a, acc_b) is one DVE op per half, PSUM -> SBUF bf16; host
  casts to f32, reassembles, and fills empty end-windows with f32 min
  (data-independent), matching the reference.
"""

import os
import sys

import numpy as np
import ml_dtypes

if os.path.isdir("/opt/trn_rl_repo") and "/opt/trn_rl_repo" not in sys.path:
    sys.path.insert(0, "/opt/trn_rl_repo")

import concourse.bass as bass  # noqa: E402
from concourse import bacc, mybir, tile  # noqa: E402
from concourse.bass_utils import run_bass_kernel_spmd  # noqa: E402

B, C2, T = 16, 512, 126
C = C2 // 2  # 256
NCORES = 8
BPC = B // NCORES  # batches per core = 2
CPRIME = BPC * C2  # 1024 columns per core
NLEV = 7
KS = [127 - (1 << k) for k in range(NLEV)]  # valid rows of level k

BF16 = mybir.dt.bfloat16
F32 = mybir.dt.float32

N_WARMUP = 5  # zero matmuls bridging kernel start -> first weights

_CACHE = {}

TRACE = False
LAST_RESULTS = None

# chunk c holds, per level k in CHUNK_LEVELS[c]: [sh_k | g(0,0,k) g(1,0,k)
# g(0,1,k) g(1,1,k)]  (sh only for k < 6)
CHUNK_LEVELS = [[0], [1, 2], [3, 4, 5, 6]]


def _wts_layout():
    """ft chunk + three weight chunks (one DMA each, ordered by level)."""
    offs = {}
    totals = []
    for levels in CHUNK_LEVELS:
        off = 0
        ci = len(totals) + 1
        for k in levels:
            if k < NLEV - 1:
                offs[("sh", k)] = (ci, off, KS[k + 1])
                off += KS[k + 1]
            for gi in range(2):
                for h in range(2):
                    offs[("g", gi, h, k)] = (ci, off, T)
                    off += T
        totals.append(off)
    return offs, totals


def _build_module():
    nc = bacc.Bacc(None, target_bir_lowering=False, debug=False)

    offs, totals = _wts_layout()
    ft = nc.dram_tensor("ft", [T, CPRIME], BF16, kind="ExternalInput")
    wts = [
        nc.dram_tensor(f"w{c}", [T, totals[c]], BF16, kind="ExternalInput")
        for c in range(3)
    ]
    out = nc.dram_tensor("out", [T, CPRIME], BF16, kind="ExternalOutput")

    with tile.TileContext(nc) as tc:
        with (
            tc.tile_pool(name="lv", bufs=1) as lvp,
            tc.tile_pool(name="gw", bufs=1) as gwp,
            tc.tile_pool(name="acc", bufs=1, space=bass.MemorySpace.PSUM) as accp,
            tc.tile_pool(name="shp", bufs=2, space=bass.MemorySpace.PSUM) as shpp,
        ):
            ftt = gwp.tile([T, CPRIME], BF16, name="ftt")
            wt = [
                gwp.tile([T, totals[c]], BF16, name=f"wt{c}") for c in range(3)
            ]
            # four queues: ft on SP, weight chunks on Pool/ACT/DVE queues,
            # ordered so level-0 weights land first.
            nc.sync.dma_start(out=ftt[:, :], in_=ft[:, :])
            nc.gpsimd.dma_start(out=wt[0][:, :], in_=wts[0][:, :])
            nc.scalar.dma_start(out=wt[1][:, :], in_=wts[1][:, :])
            nc.scalar.dma_start(out=wt[2][:, :], in_=wts[2][:, :])

            L = [ftt[:, :]] + [
                lvp.tile([KS[k], CPRIME], BF16, name=f"L{k}")[:, :]
                for k in range(1, NLEV)
            ]

            def sh_ap(k):
                ci, o, n = offs[("sh", k)]
                return wt[ci - 1][0 : KS[k], o : o + n]

            def g_ap(gi, h, k):
                ci, o, n = offs[("g", gi, h, k)]
                return wt[ci - 1][0 : KS[k], o : o + n]

            p_acc = [
                accp.tile([T, CPRIME], F32, name=f"pacc{gi}") for gi in range(2)
            ]

            # PE warmup: HAM runs the PE at reduced clock until ~3us of
            # continuous busy; zero bf16 matmuls bridge the gap between
            # kernel start and the first weight DMA landing so the real
            # matmuls run at full clock with no idle gap.
            wzero = gwp.tile([128, 512], BF16, name="wzero")
            nc.vector.memset(wzero[:, :], 0.0)
            for _ in range(N_WARMUP):
                nc.tensor.matmul(
                    p_acc[0][:, 0:512],
                    wzero[0:126, 0:126],
                    wzero[0:126, 0:512],
                    start=True,
                    stop=True,
                )

            # per level: shift both halves first (PE), gathers accumulate,
            # DVE folds the shifted rows into the next level meanwhile.
            for k in range(NLEV):
                shp = None
                if k < NLEV - 1:
                    shp = shpp.tile(
                        [KS[k + 1], CPRIME], F32, name=f"shp{k}", tag="shp"
                    )
                for h in range(2):
                    sl = slice(h * 512, (h + 1) * 512)
                    if k < NLEV - 1:
                        nc.tensor.matmul(
                            shp[:, sl],
                            sh_ap(k),
                            L[k][:, sl],
                            start=True,
                            stop=True,
                        )
                    for gi in range(2):
                        nc.tensor.matmul(
                            p_acc[gi][:, sl],
                            g_ap(gi, h, k),
                            L[k][:, sl],
                            start=(k == 0),
                            stop=(k == NLEV - 1),
                        )
                    if k < NLEV - 1:
                        nc.vector.tensor_max(
                            L[k + 1][:, sl],
                            L[k][0 : KS[k + 1], sl],
                            shp[:, sl],
                        )

            # final per half (DVE can read only one PSUM operand per op):
            # stage acc0 to SBUF bf16 via tensor_scalar_max, then max with
            # acc1; one output DMA per half.
            s1t = gwp.tile([T, CPRIME], BF16, name="s1t")
            ot = gwp.tile([T, CPRIME], BF16, name="ot")
            for half in range(2):
                sl = slice(half * 512, (half + 1) * 512)
                nc.vector.tensor_scalar_max(s1t[:, sl], p_acc[0][:, sl], -3.0e38)
                nc.vector.tensor_max(ot[:, sl], s1t[:, sl], p_acc[1][:, sl])
                eng = nc.sync if half == 0 else nc.scalar
                eng.dma_start(out=out[:, sl], in_=ot[:, sl])

    nc.compile()
    return nc


def _host_windows(segments):
    """Replicates the reference's index math on segments[0]. Returns per half
    (lo, hi) clamped windows plus the empty mask."""
    seg = np.clip(segments.astype(np.float32), 0.0, 125.0)
    row = seg[0]  # [T, 4]
    s0 = np.floor(row[:, 0]).astype(np.int32)
    s1 = np.ceil(row[:, 1]).astype(np.int32)
    s1 = np.where(s0 == s1, s1 + 1, s1)
    e0 = np.floor(row[:, 2]).astype(np.int32)
    e1 = np.ceil(row[:, 3]).astype(np.int32)
    e0 = np.where(e0 == e1, e0 - 1, e0)

    halves = []
    for lo, hi in ((s0, s1), (e0, e1)):
        lo_c = np.maximum(lo, 0)
        hi_c = np.minimum(hi, T)
        empty = lo_c >= hi_c
        halves.append((lo_c, hi_c, empty))
    return halves


def _host_matrices(segments):
    halves = _host_windows(segments)
    g = {
        (gi, h, k): np.zeros((KS[k], T), np.float32)
        for gi in range(2)
        for h in range(2)
        for k in range(NLEV)
    }
    for h, (lo, hi, empty) in enumerate(halves):
        for t in range(T):
            if empty[t]:
                continue
            ln = int(hi[t] - lo[t])
            k = ln.bit_length() - 1
            g[(0, h, k)][int(lo[t]), t] = 1.0
            g[(1, h, k)][int(hi[t]) - (1 << k), t] = 1.0
    sh = {}
    for k in range(NLEV - 1):
        m = np.zeros((KS[k], KS[k + 1]), np.float32)
        s = 1 << k
        for j in range(KS[k + 1]):
            m[j + s, j] = 1.0
        sh[k] = m
    return g, sh, halves


def _shard_feature(feature):
    """Core i gets batches [2i, 2i+2) as [T, CPRIME] bf16 with
    c' = half*512 + local_batch*256 + channel_within_half."""
    fts = []
    for i in range(NCORES):
        pair = feature[BPC * i : BPC * (i + 1)]
        arr = pair.reshape(BPC, 2, C, T)  # [b, h, c, j]
        arr = np.ascontiguousarray(arr.transpose(3, 1, 0, 2).reshape(T, CPRIME))
        fts.append(arr.astype(ml_dtypes.bfloat16))
    return fts


def _unshard(results, halves):
    out = np.empty((B, C2, T), np.float32)
    for i in range(NCORES):
        r = np.asarray(results[i]["out"]).astype(np.float32)  # [T, CPRIME]
        arr = r.reshape(T, 2, BPC, C).transpose(2, 1, 3, 0)  # [b, h, c, t]
        out[BPC * i : BPC * (i + 1)] = arr.reshape(BPC, C2, T)
    neg = np.finfo(np.float32).min
    for h, (_, _, empty) in enumerate(halves):
        if empty.any():
            out[:, h * C : (h + 1) * C, empty] = neg
    return out


def kernel(feature, segments):
    global LAST_RESULTS
    feature = np.ascontiguousarray(feature, dtype=np.float32)
    segments = np.ascontiguousarray(segments, dtype=np.float32)

    g, sh, halves = _host_matrices(segments)
    if "nc" not in _CACHE:
        _CACHE["nc"] = _build_module()
    nc = _CACHE["nc"]

    fts = _shard_feature(feature)

    offs, totals = _wts_layout()
    chunks = [np.zeros((T, totals[c]), np.float32) for c in range(3)]
    for k in range(NLEV):
        if k < NLEV - 1:
            ci, o, n = offs[("sh", k)]
            chunks[ci - 1][: KS[k], o : o + n] = sh[k]
        for gi in range(2):
            for h in range(2):
                ci, o, n = offs[("g", gi, h, k)]
                chunks[ci - 1][: KS[k], o : o + n] = g[(gi, h, k)]
    chunks = [c.astype(ml_dtypes.bfloat16) for c in chunks]
    in_maps = []
    for i in range(NCORES):
        m = {"ft": fts[i]}
        for c in range(3):
            m[f"w{c}"] = chunks[c]
        in_maps.append(m)

    res = run_bass_kernel_spmd(nc, in_maps, list(range(NCORES)), trace=TRACE)
    LAST_RESULTS = res
    return _unshard(res.results, halves)


# revision 19
# speedup vs baseline: 1.3308x; 1.1069x over previous
"""BoundaryMaxPooling Trainium2 kernel (v3: bf16 PE sparse-table).

Reference computation (B=16, C2=512, T=Tf=126):
  - segment windows [s0,s1) / [e0,e1) derived from segments[0] only
  - out[b, c, t]      = max_{j in [s0(t), s1(t))} feature[b, c, j]       (c < 256)
  - out[b, 256+c, t]  = max_{j in [e0(t), e1(t))} feature[b, 256+c, j]

Device algorithm (per core, 2 batches, data-parallel over batch), all in
bf16 (rel tol 2e-2 >> bf16's 4e-3):
  Sparse-table (log-level) range max with time j on SBUF partitions:
    L_0[j, c'] = feature^T   (c' = half*512 + b*256 + c, 1024 columns)
    L_{k+1}[j] = max(L_k[j], L_k[j + 2^k])
  Partition shifts and the two per-window lookups (columns lo and
  hi - 2^k of level floor(log2 len)) are exact one-hot bf16 matmuls on
  the PE (1 cycle/row), accumulated over levels in f32 PSUM; host
  precomputes the one-hot matrices from segments[0] (replicated across
  cores).  Weights stream over four DMA queues ordered by consumption;
  a short run of zero matmuls keeps the PE busy from kernel start so
  HAM has it at full clock when the real data lands.  The final
  max(acc_a, acc_b) is one DVE op per half, PSUM -> SBUF bf16; host
  casts to f32, reassembles, and fills empty end-windows with f32 min
  (data-independent), matching the reference.
"""

import os
import sys

import numpy as np
import ml_dtypes

if os.path.isdir("/opt/trn_rl_repo") and "/opt/trn_rl_repo" not in sys.path:
    sys.path.insert(0, "/opt/trn_rl_repo")

import concourse.bass as bass  # noqa: E402
from concourse import bacc, mybir, tile  # noqa: E402
from concourse.bass_utils import run_bass_kernel_spmd  # noqa: E402

B, C2, T = 16, 512, 126
C = C2 // 2  # 256
NCORES = 8
BPC = B // NCORES  # batches per core = 2
CPRIME = BPC * C2  # 1024 columns per core
NLEV = 7
KS = [127 - (1 << k) for k in range(NLEV)]  # valid rows of level k

BF16 = mybir.dt.bfloat16
F32 = mybir.dt.float32

N_WARMUP = 6  # zero matmuls bridging kernel start -> first weights

_CACHE = {}

TRACE = False
LAST_RESULTS = None

# chunk c holds, per level k in CHUNK_LEVELS[c]: [sh_k | g(0,0,k) g(1,0,k)
# g(0,1,k) g(1,1,k)]  (sh only for k < 6)
CHUNK_LEVELS = [[0], [1, 2], [3, 4, 5, 6]]


def _wts_layout():
    """ft chunk + three weight chunks (one DMA each, ordered by level)."""
    offs = {}
    totals = []
    for levels in CHUNK_LEVELS:
        off = 0
        ci = len(totals) + 1
        for k in levels:
            if k < NLEV - 1:
                offs[("sh", k)] = (ci, off, KS[k + 1])
                off += KS[k + 1]
            for gi in range(2):
                for h in range(2):
                    offs[("g", gi, h, k)] = (ci, off, T)
                    off += T
        totals.append(off)
    return offs, totals


def _build_module():
    nc = bacc.Bacc(None, target_bir_lowering=False, debug=False)

    offs, totals = _wts_layout()
    # level-0 weights ride in the same DMA as the feature (bigger rows
    # stream faster, and level 0 gates the first real matmul)
    ft = nc.dram_tensor("ft", [T, CPRIME + totals[0]], BF16, kind="ExternalInput")
    wts = [
        nc.dram_tensor(f"w{c}", [T, totals[c]], BF16, kind="ExternalInput")
        for c in range(1, 3)
    ]
    out = nc.dram_tensor("out", [T, CPRIME], BF16, kind="ExternalOutput")

    with tile.TileContext(nc) as tc:
        with (
            tc.tile_pool(name="lv", bufs=1) as lvp,
            tc.tile_pool(name="gw", bufs=1) as gwp,
            tc.tile_pool(name="acc", bufs=1, space=bass.MemorySpace.PSUM) as accp,
            tc.tile_pool(name="shp", bufs=2, space=bass.MemorySpace.PSUM) as shpp,
        ):
            ftw = gwp.tile([T, CPRIME + totals[0]], BF16, name="ftw")
            wt12 = [
                gwp.tile([T, totals[c]], BF16, name=f"wt{c}") for c in (1, 2)
            ]
            wt = [ftw, wt12[0], wt12[1]]  # chunk index -> tile
            woff = [CPRIME, 0, 0]  # chunk base column within its tile
            # ft + level-0 weights in one SP-queue DMA (bigger rows stream
            # faster and level 0 gates the first real matmul); w1 on the
            # ACT queue, w2 (consumed last) on the Pool queue.
            nc.sync.dma_start(out=ftw[:, :], in_=ft[:, :])
            nc.scalar.dma_start(out=wt12[0][:, :], in_=wts[0][:, :])
            nc.gpsimd.dma_start(out=wt12[1][:, :], in_=wts[1][:, :])

            L = [ftw[:, 0:CPRIME]] + [
                lvp.tile([KS[k], CPRIME], BF16, name=f"L{k}")[:, :]
                for k in range(1, NLEV)
            ]

            def sh_ap(k):
                ci, o, n = offs[("sh", k)]
                o += woff[ci - 1]
                return wt[ci - 1][0 : KS[k], o : o + n]

            def g_ap(gi, h, k):
                ci, o, n = offs[("g", gi, h, k)]
                o += woff[ci - 1]
                return wt[ci - 1][0 : KS[k], o : o + n]

            p_acc = [
                accp.tile([T, CPRIME], F32, name=f"pacc{gi}") for gi in range(2)
            ]

            # PE warmup: HAM runs the PE at reduced clock until ~3us of
            # continuous busy; zero bf16 matmuls bridge the gap between
            # kernel start and the first weight DMA landing so the real
            # matmuls run at full clock with no idle gap.
            wzero = gwp.tile([128, 512], BF16, name="wzero")
            nc.vector.memset(wzero[:, :], 0.0)
            for _ in range(N_WARMUP):
                nc.tensor.matmul(
                    p_acc[0][:, 0:512],
                    wzero[0:126, 0:126],
                    wzero[0:126, 0:512],
                    start=True,
                    stop=True,
                )

            # per level: shift both halves first (PE), gathers accumulate,
            # DVE folds the shifted rows into the next level meanwhile.
            for k in range(NLEV):
                pe_shift = k < NLEV - 1
                for h in range(2) if k < NLEV - 1 else ():
                    sl = slice(h * 512, (h + 1) * 512)
                    if pe_shift:
                        shp = shpp.tile(
                            [KS[k + 1], 512], F32, name=f"shp{k}{h}", tag="shp"
                        )
                        nc.tensor.matmul(
                            shp[:, :],
                            sh_ap(k),
                            L[k][:, sl],
                            start=True,
                            stop=True,
                        )
                    if k < NLEV - 1:
                        for gi in range(2):
                            nc.tensor.matmul(
                                p_acc[gi][:, sl],
                                g_ap(gi, h, k),
                                L[k][:, sl],
                                start=(k == 0),
                                stop=False,
                            )
                    if pe_shift:
                        nc.vector.tensor_max(
                            L[k + 1][:, sl],
                            L[k][0 : KS[k + 1], sl],
                            shp[:, :],
                        )
                if k == NLEV - 1:
                    # last level, gi-major: acc0's final writes land first so
                    # the DVE staging op can start under the acc1 passes
                    for gi in range(2):
                        for h in range(2):
                            sl = slice(h * 512, (h + 1) * 512)
                            nc.tensor.matmul(
                                p_acc[gi][:, sl],
                                g_ap(gi, h, k),
                                L[k][:, sl],
                                start=False,
                                stop=True,
                            )

            # final per half (DVE can read only one PSUM operand per op):
            # stage acc0 to SBUF bf16 via tensor_scalar_max, then max with
            # acc1; one output DMA per half.
            s1t = gwp.tile([T, CPRIME], BF16, name="s1t")
            ot = gwp.tile([T, CPRIME], BF16, name="ot")
            for half in range(2):
                sl = slice(half * 512, (half + 1) * 512)
                nc.vector.tensor_scalar_max(s1t[:, sl], p_acc[0][:, sl], -3.0e38)
                nc.vector.tensor_max(ot[:, sl], s1t[:, sl], p_acc[1][:, sl])
                eng = nc.sync if half == 0 else nc.scalar
                eng.dma_start(out=out[:, sl], in_=ot[:, sl])

    nc.compile()
    return nc


def _host_windows(segments):
    """Replicates the reference's index math on segments[0]. Returns per half
    (lo, hi) clamped windows plus the empty mask."""
    seg = np.clip(segments.astype(np.float32), 0.0, 125.0)
    row = seg[0]  # [T, 4]
    s0 = np.floor(row[:, 0]).astype(np.int32)
    s1 = np.ceil(row[:, 1]).astype(np.int32)
    s1 = np.where(s0 == s1, s1 + 1, s1)
    e0 = np.floor(row[:, 2]).astype(np.int32)
    e1 = np.ceil(row[:, 3]).astype(np.int32)
    e0 = np.where(e0 == e1, e0 - 1, e0)

    halves = []
    for lo, hi in ((s0, s1), (e0, e1)):
        lo_c = np.maximum(lo, 0)
        hi_c = np.minimum(hi, T)
        empty = lo_c >= hi_c
        halves.append((lo_c, hi_c, empty))
    return halves


def _host_matrices(segments):
    halves = _host_windows(segments)
    g = {
        (gi, h, k): np.zeros((KS[k], T), np.float32)
        for gi in range(2)
        for h in range(2)
        for k in range(NLEV)
    }
    for h, (lo, hi, empty) in enumerate(halves):
        for t in range(T):
            if empty[t]:
                continue
            ln = int(hi[t] - lo[t])
            k = ln.bit_length() - 1
            g[(0, h, k)][int(lo[t]), t] = 1.0
            g[(1, h, k)][int(hi[t]) - (1 << k), t] = 1.0
    sh = {}
    for k in range(NLEV - 1):
        m = np.zeros((KS[k], KS[k + 1]), np.float32)
        s = 1 << k
        for j in range(KS[k + 1]):
            m[j + s, j] = 1.0
        sh[k] = m
    return g, sh, halves


def _shard_feature(feature):
    """Core i gets batches [2i, 2i+2) as [T, CPRIME] bf16 with
    c' = half*512 + local_batch*256 + channel_within_half."""
    fts = []
    for i in range(NCORES):
        pair = feature[BPC * i : BPC * (i + 1)]
        arr = pair.reshape(BPC, 2, C, T)  # [b, h, c, j]
        arr = np.ascontiguousarray(arr.transpose(3, 1, 0, 2).reshape(T, CPRIME))
        fts.append(arr.astype(ml_dtypes.bfloat16))
    return fts


def _unshard(results, halves):
    out = np.empty((B, C2, T), np.float32)
    for i in range(NCORES):
        r = np.asarray(results[i]["out"]).astype(np.float32)  # [T, CPRIME]
        arr = r.reshape(T, 2, BPC, C).transpose(2, 1, 3, 0)  # [b, h, c, t]
        out[BPC * i : BPC * (i + 1)] = arr.reshape(BPC, C2, T)
    neg = np.finfo(np.float32).min
    for h, (_, _, empty) in enumerate(halves):
        if empty.any():
            out[:, h * C : (h + 1) * C, empty] = neg
    return out


def kernel(feature, segments):
    global LAST_RESULTS
    feature = np.ascontiguousarray(feature, dtype=np.float32)
    segments = np.ascontiguousarray(segments, dtype=np.float32)

    g, sh, halves = _host_matrices(segments)
    if "nc" not in _CACHE:
        _CACHE["nc"] = _build_module()
    nc = _CACHE["nc"]

    fts = _shard_feature(feature)

    offs, totals = _wts_layout()
    chunks = [np.zeros((T, totals[c]), np.float32) for c in range(3)]
    for k in range(NLEV):
        if k < NLEV - 1:
            ci, o, n = offs[("sh", k)]
            chunks[ci - 1][: KS[k], o : o + n] = sh[k]
        for gi in range(2):
            for h in range(2):
                ci, o, n = offs[("g", gi, h, k)]
                chunks[ci - 1][: KS[k], o : o + n] = g[(gi, h, k)]
    chunks = [c.astype(ml_dtypes.bfloat16) for c in chunks]
    in_maps = []
    for i in range(NCORES):
        m = {
            "ft": np.ascontiguousarray(np.concatenate([fts[i], chunks[0]], axis=1)),
            "w1": chunks[1],
            "w2": chunks[2],
        }
        in_maps.append(m)

    res = run_bass_kernel_spmd(nc, in_maps, list(range(NCORES)), trace=TRACE)
    LAST_RESULTS = res
    return _unshard(res.results, halves)
